# revision 18
# baseline (speedup 1.0000x reference)
"""GAT dual-graph kernel for 8 TRN2 NeuronCores.

dst-partitioned nodes/edges, replicated weights, AllGather'd bf16 row-tables
[h | 1 | s_src], per-edge dma_gather of 512B/256B rows, attention softmax
(max-free: scores are O(1)) folded into one-hot selection matrices, PE
matmul scatter-accumulate into 128-node PSUM windows with a ones-column
denominator, relu(agg/denom) flush. Two launches with a host hop to bake
s_dst per edge-slot (layer-2 s_dst needs layer-1 output; no per-partition
gather primitive exists on-chip).
"""

import numpy as np
import ml_dtypes

import concourse.bass as bass
import concourse.bacc as bacc
import concourse.mybir as mybir
import concourse.tile as tile
from concourse.bass_utils import run_bass_kernel_spmd

TRACE = False
TIME_RERUN = False
LAST_EXEC_NS = []
LAST_WALL_S = []

N = 50000
G = 128
D = 256
NEG = 0.2
NC = 8
NPC = N // NC
NWIN = (NPC + 127) // 128     # 49; last window has 106 nodes
TAIL = NPC - (NWIN - 1) * 128  # 106
SPLIT = 32768
HI_OFF = 17232                 # hi half rows [17232, 50000) -> 32768 rows
BF = ml_dtypes.bfloat16
F32 = mybir.dt.float32
BF16 = mybir.dt.bfloat16
I16 = mybir.dt.int16
AF = mybir.ActivationFunctionType
OP = mybir.AluOpType


NW32 = (NPC + 31) // 32   # 196 32-node windows per core


def _preprocess(src, dst):
    """Shared (max-over-cores) slot schedule + per-core slot arrays.
    Slot order per core: (win32, half, dst); group (win32, half) sizes are
    max-over-cores rounded up to 128 so every Z column is single-group."""
    loop = np.arange(N, dtype=np.int64)
    src = np.concatenate([src.astype(np.int64), loop])
    dst = np.concatenate([dst.astype(np.int64), loop])
    core = dst // NPC
    dstloc = dst - core * NPC
    win = dstloc // 32
    half = (src >= SPLIT).astype(np.int64)
    gid = win * 2 + half
    ngroups = NW32 * 2
    counts = np.zeros((NC, ngroups), dtype=np.int64)
    np.add.at(counts, (core, gid), 1)
    gsize = counts.max(axis=0)
    gsize = ((gsize + 127) // 128) * 128
    goff = np.zeros(ngroups + 1, dtype=np.int64)
    np.cumsum(gsize, out=goff[1:])
    nslot = int(goff[-1])

    idx16 = np.zeros((NC, nslot), dtype=np.int16)
    posrel = np.full((NC, nslot), -1.0, dtype=np.float32)
    dstslot = np.zeros((NC, nslot), dtype=np.int64)

    order = np.lexsort((dst, half, win, core))
    src_o, dst_o, core_o, gid_o, half_o, dstloc_o = (
        src[order], dst[order], core[order], gid[order], half[order],
        dstloc[order])
    keys = core_o * ngroups + gid_o
    _, first_idx, inv = np.unique(keys, return_index=True, return_inverse=True)
    pos_in_g = np.arange(len(order)) - first_idx[inv]
    slot = goff[gid_o] + pos_in_g
    idxv = np.where(half_o == 0, src_o, src_o - HI_OFF)
    idx16[core_o, slot] = idxv.astype(np.int16)
    posrel[core_o, slot] = (dstloc_o % 32).astype(np.float32)
    dstslot[core_o, slot] = dst_o

    # batches of groups -> one dma_gather each; columns annotated with
    # (win32, half, first/last-of-window)
    cols = []   # (win32, half)
    for g in range(ngroups):
        w, h = divmod(g, 2)
        cols += [(w, h)] * (int(gsize[g]) // 128)
    ncols = nslot // 128
    first_col = {}
    last_col = {}
    for ci, (w, h) in enumerate(cols):
        first_col.setdefault(w, ci)
        last_col[w] = ci
    batches = []  # (col_off, ncols_batch, half)
    BCOL = 16
    co = 0
    while co < ncols:
        h = cols[co][1]
        bc = 1
        while (bc < BCOL and co + bc < ncols and cols[co + bc][1] == h):
            bc += 1
        batches.append((co, bc, h))
        co += bc
    return dict(idx16=idx16, posrel=posrel, dstslot=dstslot,
                cols=cols, first_col=first_col, last_col=last_col,
                batches=batches, nslot=nslot)


def _slot_pc(a):
    return np.ascontiguousarray(a.reshape(-1, 128).T)


def _wrap16(a):
    return np.tile(np.ascontiguousarray(a.reshape(-1, 16).T), (8, 1))


def _edge_phase(nc, sbuf, psum, pp, R, NU, z_lo, z_hi, idx_sb,
                pos_sb, v_sb, iota_sb, xout_sb, FOUT, tag):
    """Gather + attention + PE scatter for one (layer, side)."""
    cols = pp['cols']
    first_col, last_col = pp['first_col'], pp['last_col']
    live = {}
    for (co, bc, h) in pp['batches']:
        n = bc * 128
        off = co * 128
        z = sbuf.tile([128, bc, R], BF16, tag=f"z{tag}")
        nc.gpsimd.dma_gather(
            z[:, 0:bc, :], (z_lo if h == 0 else z_hi),
            idx_sb[:, off // 16:(off + n) // 16], n, n, R)
        t = sbuf.tile([128, bc], F32, tag=f"t{tag}")
        nc.vector.tensor_tensor(out=t[:, 0:bc], in0=z[:, 0:bc, NU],
                                in1=v_sb[:, co:co + bc], op=OP.add)
        t2 = sbuf.tile([128, bc], F32, tag=f"t2{tag}")
        nc.vector.tensor_scalar_mul(out=t2[:, 0:bc], in0=t[:, 0:bc],
                                    scalar1=NEG)
        nc.vector.tensor_tensor(out=t[:, 0:bc], in0=t[:, 0:bc],
                                in1=t2[:, 0:bc], op=OP.max)
        wexp = sbuf.tile([128, bc], BF16, tag=f"w{tag}")
        nc.scalar.activation(wexp[:, 0:bc], t[:, 0:bc], AF.Exp)
        sel = sbuf.tile([128, 32, bc], BF16, tag=f"sel{tag}")
        nc.vector.tensor_tensor(
            out=sel[:, :, 0:bc], in0=iota_sb[:, :, 0:bc],
            in1=pos_sb[:, co:co + bc].rearrange(
                "p (a c) -> p a c", a=1).to_broadcast([128, 32, bc]),
            op=OP.is_equal)
        nc.vector.tensor_tensor(
            out=sel[:, :, 0:bc], in0=sel[:, :, 0:bc],
            in1=wexp[:, 0:bc].rearrange(
                "p (a c) -> p a c", a=1).to_broadcast([128, 32, bc]),
            op=OP.mult)
        for cl in range(bc):
            ci = co + cl
            w = cols[ci][0]
            if w not in live:
                live[w] = psum.tile([32, NU + 1], F32, tag="pw",
                                    name="pw")
            nc.tensor.matmul(
                out=live[w][:, 0:NU + 1],
                lhsT=sel[:, :, cl],
                rhs=z[:, cl, 0:NU + 1],
                start=(ci == first_col[w]), stop=(ci == last_col[w]))
            if ci == last_col[w]:
                pw = live.pop(w)
                rec = sbuf.tile([32, 1], F32, tag=f"rec{tag}")
                nc.vector.reciprocal(rec[:, :], pw[:, NU - 1:NU])
                xtmp = sbuf.tile([32, FOUT], BF16, tag=f"xt{tag}")
                nc.scalar.activation(xtmp[:, :], pw[:, 0:FOUT],
                                     AF.Relu, scale=rec[:, :])
                pb = 32 * (w % 4)
                tr = (NPC - (NW32 - 1) * 32) if w == NW32 - 1 else 32
                nc.sync.dma_start(xout_sb[pb:pb + tr, w // 4, 0:FOUT],
                                  xtmp[0:tr, :])


def _store_rows(nc, dram_t, sb_tile, ncols):
    """sbuf [128, NWIN, C] (node=(w*128+p)) -> DRAM [NPC, C]."""
    nc.sync.dma_start(
        dram_t[0:(NWIN - 1) * 128, :].rearrange("(w p) c -> p w c", p=128),
        sb_tile[:, 0:NWIN - 1, 0:ncols])
    nc.sync.dma_start(dram_t[(NWIN - 1) * 128:NPC, :],
                      sb_tile[0:TAIL, NWIN - 1, 0:ncols])


def _build_launch1(pps, cwmax):
    nc = bacc.Bacc("TRN2", target_bir_lowering=False, debug=False,
                   num_devices=NC)
    dram_in = lambda n, sh, dt: nc.dram_tensor(n, sh, dt, kind="ExternalInput")
    xT = {s: dram_in(f"xT_{s}", [D, NPC], BF16) for s in "st"}
    Wa = {s: dram_in(f"W1a_{s}", [D, 130], BF16) for s in "st"}
    idx = {s: dram_in(f"idx_{s}", [128, pps[s]['nslot'] // 16], I16) for s in "st"}
    pos = {s: dram_in(f"pos_{s}", [128, pps[s]['nslot'] // 128], BF16) for s in "st"}
    v1 = {s: dram_in(f"v1_{s}", [128, pps[s]['nslot'] // 128], BF16) for s in "st"}
    iota = dram_in("iota", [128, 32, cwmax], BF16)
    xout = {s: nc.dram_tensor(f"x2_{s}", [NPC, 128], BF16,
                              kind="ExternalOutput") for s in "st"}

    with tile.TileContext(nc) as tc:
        with tc.tile_pool(name="sb", bufs=2) as sbuf, \
             tc.tile_pool(name="sb1", bufs=1) as sb1, \
             tc.tile_pool(name="ps", bufs=2, space="PSUM") as psum, \
             tc.tile_pool(name="dram", bufs=1, space="DRAM") as dram:
            iota_sb = sb1.tile([128, 32, cwmax], BF16)
            nc.sync.dma_start(iota_sb[:, :, :], iota[:, :, :])
            for s in "st":
                pp = pps[s]
                ns = pp['nslot']
                idx_sb = sb1.tile([128, ns // 16], I16, tag=f"idx{s}")
                nc.sync.dma_start(idx_sb[:, :], idx[s][:, :])
                pos_sb = sb1.tile([128, ns // 128], BF16, tag=f"pos{s}")
                nc.sync.dma_start(pos_sb[:, :], pos[s][:, :])
                v_sb = sb1.tile([128, ns // 128], BF16, tag=f"v{s}")
                nc.sync.dma_start(v_sb[:, :], v1[s][:, :])
                wa_sb = sb1.tile([128, 2, 130], BF16, tag=f"wa{s}")
                for k in range(2):
                    nc.sync.dma_start(wa_sb[:, k, :],
                                      Wa[s][k * 128:(k + 1) * 128, :])
                haug = sb1.tile([128, NWIN, 256], BF16, tag=f"ha{s}")
                for w in range(NWIN):
                    m = min(128, NPC - w * 128)
                    xtw = sbuf.tile([128, 2, 128], BF16, tag="xtw")
                    for k in range(2):
                        nc.sync.dma_start(
                            xtw[:, k, 0:m],
                            xT[s][k * 128:(k + 1) * 128,
                                  w * 128:w * 128 + m])
                    ph = psum.tile([128, 130], F32, tag="phd")
                    for k in range(2):
                        nc.tensor.matmul(
                            out=ph[0:m, 0:130], lhsT=xtw[:, k, 0:m],
                            rhs=wa_sb[:, k, 0:130],
                            start=(k == 0), stop=(k == 1))
                    nc.scalar.activation(haug[0:m, w, 0:130],
                                         ph[0:m, 0:130], AF.Copy)
                nc.vector.memset(haug[:, :, 128:129], 1.0)
                hloc = dram.tile([NPC, 256], BF16, tag=f"hl{s}")
                full = dram.tile([N, 256], BF16, tag=f"hf{s}")
                _store_rows(nc, hloc, haug, 256)
                nc.gpsimd.collective_compute(
                    "AllGather", OP.bypass,
                    replica_groups=[list(range(NC))],
                    ins=[hloc.opt()], outs=[full.opt()])
                x2 = sb1.tile([128, NWIN, 128], BF16, tag=f"x2{s}")
                _edge_phase(nc, sbuf, psum, pp, 256, 129,
                            full[0:SPLIT, :], full[HI_OFF:N, :],
                            idx_sb, pos_sb, v_sb, iota_sb, x2, 128, s)
                _store_rows(nc, xout[s], x2, 128)
    nc.compile()
    return nc


def _build_launch2(pps, cwmax):
    nc = bacc.Bacc("TRN2", target_bir_lowering=False, debug=False,
                   num_devices=NC)
    dram_in = lambda n, sh, dt: nc.dram_tensor(n, sh, dt, kind="ExternalInput")
    x2T = {s: dram_in(f"x2T_{s}", [128, NPC], BF16) for s in "st"}
    Wa = {s: dram_in(f"W2a_{s}", [128, 66], BF16) for s in "st"}
    idx = {s: dram_in(f"idx_{s}", [128, pps[s]['nslot'] // 16], I16) for s in "st"}
    pos = {s: dram_in(f"pos_{s}", [128, pps[s]['nslot'] // 128], BF16) for s in "st"}
    v2 = {s: dram_in(f"v2_{s}", [128, pps[s]['nslot'] // 128], BF16) for s in "st"}
    iota = dram_in("iota", [128, 32, cwmax], BF16)
    p1hot = {s: dram_in(f"p1hot_{s}", [128, NWIN * G], BF16) for s in "st"}
    ident = dram_in("ident", [128, 128], F32)
    wlin = dram_in("wlin", [64, 128], BF16)
    out = nc.dram_tensor("out", [G, 128], F32, kind="ExternalOutput")

    with tile.TileContext(nc) as tc:
        with tc.tile_pool(name="sb", bufs=2) as sbuf, \
             tc.tile_pool(name="sb1", bufs=1) as sb1, \
             tc.tile_pool(name="ps", bufs=2, space="PSUM") as psum, \
             tc.tile_pool(name="pp", bufs=1, space="PSUM") as psum1, \
             tc.tile_pool(name="dram", bufs=1, space="DRAM") as dram:
            iota_sb = sb1.tile([128, 32, cwmax], BF16)
            nc.sync.dma_start(iota_sb[:, :, :], iota[:, :, :])
            poolcat = sb1.tile([128, 128], F32)
            for si, s in enumerate("st"):
                pp = pps[s]
                ns = pp['nslot']
                idx_sb = sb1.tile([128, ns // 16], I16, tag=f"idx{s}")
                nc.sync.dma_start(idx_sb[:, :], idx[s][:, :])
                pos_sb = sb1.tile([128, ns // 128], BF16, tag=f"pos{s}")
                nc.sync.dma_start(pos_sb[:, :], pos[s][:, :])
                v_sb = sb1.tile([128, ns // 128], BF16, tag=f"v{s}")
                nc.sync.dma_start(v_sb[:, :], v2[s][:, :])
                wa_sb = sb1.tile([128, 66], BF16, tag=f"wa{s}")
                nc.sync.dma_start(wa_sb[:, :], Wa[s][:, :])
                x2T_sb = sb1.tile([128, NPC], BF16, tag=f"xT{s}")
                nc.sync.dma_start(x2T_sb[:, :], x2T[s][:, :])
                haug = sb1.tile([128, NWIN, 128], BF16, tag=f"ha{s}")
                for w in range(NWIN):
                    m = min(128, NPC - w * 128)
                    ph = psum.tile([128, 66], F32, tag="phd")
                    nc.tensor.matmul(
                        out=ph[0:m, 0:66],
                        lhsT=x2T_sb[:, w * 128:w * 128 + m],
                        rhs=wa_sb[:, 0:66], start=True, stop=True)
                    nc.scalar.activation(haug[0:m, w, 0:66],
                                         ph[0:m, 0:66], AF.Copy)
                nc.vector.memset(haug[:, :, 64:65], 1.0)
                hloc = dram.tile([NPC, 128], BF16, tag=f"hl{s}")
                full = dram.tile([N, 128], BF16, tag=f"hf{s}")
                _store_rows(nc, hloc, haug, 128)
                nc.gpsimd.collective_compute(
                    "AllGather", OP.bypass,
                    replica_groups=[list(range(NC))],
                    ins=[hloc.opt()], outs=[full.opt()])
                x4 = sb1.tile([128, NWIN, 64], BF16, tag=f"x4{s}")
                nc.vector.memset(x4[96:128, NWIN - 1, :], 0.0)
                _edge_phase(nc, sbuf, psum, pp, 128, 65,
                            full[0:SPLIT, :], full[HI_OFF:N, :],
                            idx_sb, pos_sb, v_sb, iota_sb, x4, 64, s)
                ph_sb = sb1.tile([128, NWIN * G], BF16, tag=f"p1h{s}")
                nc.sync.dma_start(ph_sb[:, :], p1hot[s][:, :])
                pl = psum1.tile([128, 64], F32, tag=f"pool{s}")
                for w in range(NWIN):
                    nc.tensor.matmul(
                        out=pl[:, 0:64],
                        lhsT=ph_sb[:, w * G:(w + 1) * G],
                        rhs=x4[:, w, 0:64],
                        start=(w == 0), stop=(w == NWIN - 1))
                nc.vector.tensor_copy(out=poolcat[:, si * 64:si * 64 + 64],
                                      in_=pl[:, 0:64])
            pin = dram.tile([128, 128], F32, tag="pin")
            pout = dram.tile([128, 128], F32, tag="pout")
            nc.sync.dma_start(pin[:, :], poolcat[:, :])
            nc.gpsimd.collective_compute(
                "AllReduce", OP.add, replica_groups=[list(range(NC))],
                ins=[pin.opt()], outs=[pout.opt()])
            pred = sb1.tile([128, 128], F32)
            nc.sync.dma_start(pred[:, :], pout[:, :])
            pg = sb1.tile([128, 64], F32)
            nc.vector.tensor_tensor(out=pg[:, :], in0=pred[:, 0:64],
                                    in1=pred[:, 64:128], op=OP.add)
            id_sb = sb1.tile([128, 128], F32)
            nc.sync.dma_start(id_sb[:, :], ident[:, :])
            pT_ps = psum1.tile([64, 128], F32, tag="pT")
            nc.tensor.transpose(out=pT_ps[:, :], in_=pg[:, :],
                                identity=id_sb[:, :])
            pT = sb1.tile([64, 128], BF16)
            nc.vector.tensor_copy(out=pT[:, :], in_=pT_ps[:, :])
            wl_sb = sb1.tile([64, 128], BF16)
            nc.sync.dma_start(wl_sb[:, :], wlin[:, :])
            oph = psum1.tile([128, 128], F32, tag="pT", name="oph")
            nc.tensor.matmul(out=oph[:, :], lhsT=pT[:, :], rhs=wl_sb[:, :],
                             start=True, stop=True)
            osb = sb1.tile([128, 128], F32)
            nc.scalar.activation(osb[:, :], oph[:, :], AF.Sigmoid)
            nc.sync.dma_start(out[:, :], osb[:, :])
    nc.compile()
    return nc


def kernel(x_s, x_t, edge_index_s, edge_index_t, xs_batch, xt_batch,
           Ws1, as1_src, as1_dst, bs1, Ws2, as2_src, as2_dst, bs2,
           Wt1, at1_src, at1_dst, bt1, Wt2, at2_src, at2_dst, bt2,
           Wlin, blin):
    for b in (bs1, bs2, bt1, bt2, blin):
        assert not np.any(np.asarray(b)), "nonzero bias unsupported"
    x = {"s": np.asarray(x_s, np.float32), "t": np.asarray(x_t, np.float32)}
    W1 = {"s": np.asarray(Ws1, np.float32), "t": np.asarray(Wt1, np.float32)}
    a1s = {"s": np.asarray(as1_src, np.float32),
           "t": np.asarray(at1_src, np.float32)}
    a1d = {"s": np.asarray(as1_dst, np.float32),
           "t": np.asarray(at1_dst, np.float32)}
    W2 = {"s": np.asarray(Ws2, np.float32), "t": np.asarray(Wt2, np.float32)}
    a2s = {"s": np.asarray(as2_src, np.float32),
           "t": np.asarray(at2_src, np.float32)}
    a2d = {"s": np.asarray(as2_dst, np.float32),
           "t": np.asarray(at2_dst, np.float32)}
    batch = {"s": np.asarray(xs_batch), "t": np.asarray(xt_batch)}
    ei = {"s": np.asarray(edge_index_s), "t": np.asarray(edge_index_t)}

    pps = {s: _preprocess(ei[s][0], ei[s][1]) for s in "st"}
    cwmax = max(max(b[1] for b in pps[s]['batches']) for s in "st")
    iota_np = np.ascontiguousarray(np.broadcast_to(
        np.arange(32, dtype=np.float32)[None, :, None],
        (128, 32, cwmax))).astype(BF)

    in_maps1 = []
    for c in range(NC):
        m = {"iota": iota_np}
        for s in "st":
            m[f"xT_{s}"] = np.ascontiguousarray(
                x[s][c * NPC:(c + 1) * NPC, :].T).astype(BF)
            wa = np.zeros((D, 130), np.float32)
            wa[:, 0:128] = W1[s]
            wa[:, 129] = W1[s] @ a1s[s]
            m[f"W1a_{s}"] = wa.astype(BF)
            m[f"idx_{s}"] = _wrap16(pps[s]['idx16'][c])
            m[f"pos_{s}"] = _slot_pc(pps[s]['posrel'][c]).astype(BF)
            sdst = x[s] @ (W1[s] @ a1d[s])
            m[f"v1_{s}"] = _slot_pc(sdst[pps[s]['dstslot'][c]]).astype(BF)
        in_maps1.append(m)

    nc1 = _build_launch1(pps, cwmax)
    res1 = run_bass_kernel_spmd(nc1, in_maps1, core_ids=list(range(NC)),
                                trace=TRACE)
    LAST_EXEC_NS.append(res1.exec_time_ns)
    if TIME_RERUN:
        import time as _t
        t0 = _t.time()
        run_bass_kernel_spmd(nc1, in_maps1, core_ids=list(range(NC)))
        LAST_WALL_S.append(_t.time() - t0)
    x2 = {s: np.concatenate(
        [res1.results[c][f"x2_{s}"].astype(np.float32) for c in range(NC)],
        axis=0) for s in "st"}

    p1hot = {}
    for s in "st":
        cnt = np.maximum(
            np.bincount(batch[s], minlength=G).astype(np.float32), 1.0)
        oh = {}
        for c in range(NC):
            bl = batch[s][c * NPC:(c + 1) * NPC]
            mat = np.zeros((NWIN * 128, G), np.float32)
            mat[np.arange(NPC), bl] = 1.0 / cnt[bl]
            oh[c] = np.ascontiguousarray(
                mat.reshape(NWIN, 128, G).transpose(1, 0, 2)
                .reshape(128, NWIN * G)).astype(BF)
        p1hot[s] = oh

    in_maps2 = []
    for c in range(NC):
        m = {"iota": iota_np,
             "ident": np.eye(128, dtype=np.float32),
             "wlin": np.ascontiguousarray(
                 np.asarray(Wlin, np.float32)[:, c * 128:(c + 1) * 128]
             ).astype(BF)}
        for s in "st":
            m[f"x2T_{s}"] = np.ascontiguousarray(
                x2[s][c * NPC:(c + 1) * NPC, :].T).astype(BF)
            wa = np.zeros((128, 66), np.float32)
            wa[:, 0:64] = W2[s]
            wa[:, 65] = W2[s] @ a2s[s]
            m[f"W2a_{s}"] = wa.astype(BF)
            m[f"idx_{s}"] = _wrap16(pps[s]['idx16'][c])
            m[f"pos_{s}"] = _slot_pc(pps[s]['posrel'][c]).astype(BF)
            sdst2 = x2[s] @ (W2[s] @ a2d[s])
            m[f"v2_{s}"] = _slot_pc(sdst2[pps[s]['dstslot'][c]]).astype(BF)
            m[f"p1hot_{s}"] = p1hot[s][c]
        in_maps2.append(m)

    nc2 = _build_launch2(pps, cwmax)
    res2 = run_bass_kernel_spmd(nc2, in_maps2, core_ids=list(range(NC)),
                                trace=TRACE)
    LAST_EXEC_NS.append(res2.exec_time_ns)
    if TIME_RERUN:
        import time as _t
        t0 = _t.time()
        run_bass_kernel_spmd(nc2, in_maps2, core_ids=list(range(NC)))
        LAST_WALL_S.append(_t.time() - t0)
    out = np.concatenate([res2.results[c]["out"] for c in range(NC)], axis=1)
    return out.astype(np.float32)


# revision 19
# speedup vs baseline: 1.0661x; 1.0661x over previous
"""GAT dual-graph kernel for 8 TRN2 NeuronCores.

dst-partitioned nodes/edges, replicated weights, AllGather'd bf16 row-tables
[h | 1 | s_src], per-edge dma_gather of 512B/256B rows, attention softmax
(max-free: scores are O(1)) folded into one-hot selection matrices, PE
matmul scatter-accumulate into 128-node PSUM windows with a ones-column
denominator, relu(agg/denom) flush. Two launches with a host hop to bake
s_dst per edge-slot (layer-2 s_dst needs layer-1 output; no per-partition
gather primitive exists on-chip).
"""

import numpy as np
import ml_dtypes

import concourse.bass as bass
import concourse.bacc as bacc
import concourse.mybir as mybir
import concourse.tile as tile
from concourse.bass_utils import run_bass_kernel_spmd

TRACE = False
TIME_RERUN = False
LAST_EXEC_NS = []
LAST_WALL_S = []

N = 50000
G = 128
D = 256
NEG = 0.2
NC = 8
NPC = N // NC
NWIN = (NPC + 127) // 128     # 49; last window has 106 nodes
TAIL = NPC - (NWIN - 1) * 128  # 106
SPLIT = 32768
HI_OFF = 17232                 # hi half rows [17232, 50000) -> 32768 rows
BF = ml_dtypes.bfloat16
F32 = mybir.dt.float32
BF16 = mybir.dt.bfloat16
I16 = mybir.dt.int16
AF = mybir.ActivationFunctionType
OP = mybir.AluOpType


NW32 = (NPC + 31) // 32   # 196 32-node windows per core


def _preprocess(src, dst):
    """Shared (max-over-cores) slot schedule + per-core slot arrays.
    Slot order per core: (win32, half, dst); group (win32, half) sizes are
    max-over-cores rounded up to 128 so every Z column is single-group."""
    loop = np.arange(N, dtype=np.int64)
    src = np.concatenate([src.astype(np.int64), loop])
    dst = np.concatenate([dst.astype(np.int64), loop])
    core = dst // NPC
    dstloc = dst - core * NPC
    win = dstloc // 32
    half = (src >= SPLIT).astype(np.int64)
    gid = win * 2 + half
    ngroups = NW32 * 2
    counts = np.zeros((NC, ngroups), dtype=np.int64)
    np.add.at(counts, (core, gid), 1)
    gsize = counts.max(axis=0)
    gsize = ((gsize + 127) // 128) * 128
    goff = np.zeros(ngroups + 1, dtype=np.int64)
    np.cumsum(gsize, out=goff[1:])
    nslot = int(goff[-1])

    idx16 = np.zeros((NC, nslot), dtype=np.int16)
    posrel = np.full((NC, nslot), -1.0, dtype=np.float32)
    dstslot = np.zeros((NC, nslot), dtype=np.int64)

    order = np.lexsort((dst, half, win, core))
    src_o, dst_o, core_o, gid_o, half_o, dstloc_o = (
        src[order], dst[order], core[order], gid[order], half[order],
        dstloc[order])
    keys = core_o * ngroups + gid_o
    _, first_idx, inv = np.unique(keys, return_index=True, return_inverse=True)
    pos_in_g = np.arange(len(order)) - first_idx[inv]
    slot = goff[gid_o] + pos_in_g
    idxv = np.where(half_o == 0, src_o, src_o - HI_OFF)
    idx16[core_o, slot] = idxv.astype(np.int16)
    posrel[core_o, slot] = (dstloc_o % 32).astype(np.float32)
    dstslot[core_o, slot] = dst_o

    # batches of groups -> one dma_gather each; columns annotated with
    # (win32, half, first/last-of-window)
    cols = []   # (win32, half)
    for g in range(ngroups):
        w, h = divmod(g, 2)
        cols += [(w, h)] * (int(gsize[g]) // 128)
    ncols = nslot // 128
    first_col = {}
    last_col = {}
    for ci, (w, h) in enumerate(cols):
        first_col.setdefault(w, ci)
        last_col[w] = ci
    batches = []  # (col_off, ncols_batch, half)
    BCOL = 16
    co = 0
    while co < ncols:
        h = cols[co][1]
        bc = 1
        while (bc < BCOL and co + bc < ncols and cols[co + bc][1] == h):
            bc += 1
        batches.append((co, bc, h))
        co += bc
    return dict(idx16=idx16, posrel=posrel, dstslot=dstslot,
                cols=cols, first_col=first_col, last_col=last_col,
                batches=batches, nslot=nslot)


def _slot_pc(a):
    return np.ascontiguousarray(a.reshape(-1, 128).T)


def _wrap16(a):
    return np.tile(np.ascontiguousarray(a.reshape(-1, 16).T), (8, 1))


def _edge_phase(nc, sbuf, psum, pp, R, NU, z_lo, z_hi, idx_sb,
                pos_sb, v_sb, iota_sb, xout_sb, FOUT, tag):
    """Gather + attention + PE scatter for one (layer, side)."""
    cols = pp['cols']
    first_col, last_col = pp['first_col'], pp['last_col']
    live = {}
    for bi, (co, bc, h) in enumerate(pp['batches']):
        n = bc * 128
        off = co * 128
        z = sbuf.tile([128, bc, R], BF16, tag=f"z{tag}")
        nc.gpsimd.dma_gather(
            z[:, 0:bc, :], (z_lo if h == 0 else z_hi),
            idx_sb[:, off // 16:(off + n) // 16], n, n, R,
            queue_num=bi % 4)
        t = sbuf.tile([128, bc], F32, tag=f"t{tag}")
        nc.vector.tensor_tensor(out=t[:, 0:bc], in0=z[:, 0:bc, NU],
                                in1=v_sb[:, co:co + bc], op=OP.add)
        t2 = sbuf.tile([128, bc], F32, tag=f"t2{tag}")
        nc.vector.tensor_scalar_mul(out=t2[:, 0:bc], in0=t[:, 0:bc],
                                    scalar1=NEG)
        nc.vector.tensor_tensor(out=t[:, 0:bc], in0=t[:, 0:bc],
                                in1=t2[:, 0:bc], op=OP.max)
        wexp = sbuf.tile([128, bc], BF16, tag=f"w{tag}")
        nc.scalar.activation(wexp[:, 0:bc], t[:, 0:bc], AF.Exp)
        sel = sbuf.tile([128, 32, bc], BF16, tag=f"sel{tag}")
        nc.vector.tensor_tensor(
            out=sel[:, :, 0:bc], in0=iota_sb[:, :, 0:bc],
            in1=pos_sb[:, co:co + bc].rearrange(
                "p (a c) -> p a c", a=1).to_broadcast([128, 32, bc]),
            op=OP.is_equal)
        nc.vector.tensor_tensor(
            out=sel[:, :, 0:bc], in0=sel[:, :, 0:bc],
            in1=wexp[:, 0:bc].rearrange(
                "p (a c) -> p a c", a=1).to_broadcast([128, 32, bc]),
            op=OP.mult)
        for cl in range(bc):
            ci = co + cl
            w = cols[ci][0]
            if w not in live:
                live[w] = psum.tile([32, NU + 1], F32, tag="pw",
                                    name="pw")
            nc.tensor.matmul(
                out=live[w][:, 0:NU + 1],
                lhsT=sel[:, :, cl],
                rhs=z[:, cl, 0:NU + 1],
                start=(ci == first_col[w]), stop=(ci == last_col[w]))
            if ci == last_col[w]:
                pw = live.pop(w)
                rec = sbuf.tile([32, 1], F32, tag=f"rec{tag}")
                nc.vector.reciprocal(rec[:, :], pw[:, NU - 1:NU])
                xtmp = sbuf.tile([32, FOUT], BF16, tag=f"xt{tag}")
                nc.scalar.activation(xtmp[:, :], pw[:, 0:FOUT],
                                     AF.Relu, scale=rec[:, :])
                pb = 32 * (w % 4)
                tr = (NPC - (NW32 - 1) * 32) if w == NW32 - 1 else 32
                nc.sync.dma_start(xout_sb[pb:pb + tr, w // 4, 0:FOUT],
                                  xtmp[0:tr, :])


def _store_rows(nc, dram_t, sb_tile, ncols):
    """sbuf [128, NWIN, C] (node=(w*128+p)) -> DRAM [NPC, C]."""
    nc.sync.dma_start(
        dram_t[0:(NWIN - 1) * 128, :].rearrange("(w p) c -> p w c", p=128),
        sb_tile[:, 0:NWIN - 1, 0:ncols])
    nc.sync.dma_start(dram_t[(NWIN - 1) * 128:NPC, :],
                      sb_tile[0:TAIL, NWIN - 1, 0:ncols])


def _build_launch1(pps, cwmax):
    nc = bacc.Bacc("TRN2", target_bir_lowering=False, debug=False,
                   num_devices=NC, num_swdge_queues=4)
    dram_in = lambda n, sh, dt: nc.dram_tensor(n, sh, dt, kind="ExternalInput")
    xT = {s: dram_in(f"xT_{s}", [D, NPC], BF16) for s in "st"}
    Wa = {s: dram_in(f"W1a_{s}", [D, 130], BF16) for s in "st"}
    idx = {s: dram_in(f"idx_{s}", [128, pps[s]['nslot'] // 16], I16) for s in "st"}
    pos = {s: dram_in(f"pos_{s}", [128, pps[s]['nslot'] // 128], BF16) for s in "st"}
    v1 = {s: dram_in(f"v1_{s}", [128, pps[s]['nslot'] // 128], BF16) for s in "st"}
    iota = dram_in("iota", [128, 32, cwmax], BF16)
    xout = {s: nc.dram_tensor(f"x2_{s}", [NPC, 128], BF16,
                              kind="ExternalOutput") for s in "st"}

    with tile.TileContext(nc) as tc:
        with tc.tile_pool(name="sb", bufs=2) as sbuf, \
             tc.tile_pool(name="sb1", bufs=1) as sb1, \
             tc.tile_pool(name="ps", bufs=2, space="PSUM") as psum, \
             tc.tile_pool(name="dram", bufs=1, space="DRAM") as dram:
            iota_sb = sb1.tile([128, 32, cwmax], BF16)
            nc.sync.dma_start(iota_sb[:, :, :], iota[:, :, :])
            for s in "st":
                pp = pps[s]
                ns = pp['nslot']
                idx_sb = sb1.tile([128, ns // 16], I16, tag=f"idx{s}")
                nc.sync.dma_start(idx_sb[:, :], idx[s][:, :])
                pos_sb = sb1.tile([128, ns // 128], BF16, tag=f"pos{s}")
                nc.sync.dma_start(pos_sb[:, :], pos[s][:, :])
                v_sb = sb1.tile([128, ns // 128], BF16, tag=f"v{s}")
                nc.sync.dma_start(v_sb[:, :], v1[s][:, :])
                wa_sb = sb1.tile([128, 2, 130], BF16, tag=f"wa{s}")
                for k in range(2):
                    nc.sync.dma_start(wa_sb[:, k, :],
                                      Wa[s][k * 128:(k + 1) * 128, :])
                haug = sb1.tile([128, NWIN, 256], BF16, tag=f"ha{s}")
                for w in range(NWIN):
                    m = min(128, NPC - w * 128)
                    xtw = sbuf.tile([128, 2, 128], BF16, tag="xtw")
                    for k in range(2):
                        nc.sync.dma_start(
                            xtw[:, k, 0:m],
                            xT[s][k * 128:(k + 1) * 128,
                                  w * 128:w * 128 + m])
                    ph = psum.tile([128, 130], F32, tag="phd")
                    for k in range(2):
                        nc.tensor.matmul(
                            out=ph[0:m, 0:130], lhsT=xtw[:, k, 0:m],
                            rhs=wa_sb[:, k, 0:130],
                            start=(k == 0), stop=(k == 1))
                    nc.scalar.activation(haug[0:m, w, 0:130],
                                         ph[0:m, 0:130], AF.Copy)
                nc.vector.memset(haug[:, :, 128:129], 1.0)
                hloc = dram.tile([NPC, 256], BF16, tag=f"hl{s}")
                full = dram.tile([N, 256], BF16, tag=f"hf{s}")
                _store_rows(nc, hloc, haug, 256)
                nc.gpsimd.collective_compute(
                    "AllGather", OP.bypass,
                    replica_groups=[list(range(NC))],
                    ins=[hloc.opt()], outs=[full.opt()])
                x2 = sb1.tile([128, NWIN, 128], BF16, tag=f"x2{s}")
                _edge_phase(nc, sbuf, psum, pp, 256, 129,
                            full[0:SPLIT, :], full[HI_OFF:N, :],
                            idx_sb, pos_sb, v_sb, iota_sb, x2, 128, s)
                _store_rows(nc, xout[s], x2, 128)
    nc.compile()
    return nc


def _build_launch2(pps, cwmax):
    nc = bacc.Bacc("TRN2", target_bir_lowering=False, debug=False,
                   num_devices=NC, num_swdge_queues=4)
    dram_in = lambda n, sh, dt: nc.dram_tensor(n, sh, dt, kind="ExternalInput")
    x2T = {s: dram_in(f"x2T_{s}", [128, NPC], BF16) for s in "st"}
    Wa = {s: dram_in(f"W2a_{s}", [128, 66], BF16) for s in "st"}
    idx = {s: dram_in(f"idx_{s}", [128, pps[s]['nslot'] // 16], I16) for s in "st"}
    pos = {s: dram_in(f"pos_{s}", [128, pps[s]['nslot'] // 128], BF16) for s in "st"}
    v2 = {s: dram_in(f"v2_{s}", [128, pps[s]['nslot'] // 128], BF16) for s in "st"}
    iota = dram_in("iota", [128, 32, cwmax], BF16)
    p1hot = {s: dram_in(f"p1hot_{s}", [128, NWIN * G], BF16) for s in "st"}
    ident = dram_in("ident", [128, 128], F32)
    wlin = dram_in("wlin", [64, 128], BF16)
    out = nc.dram_tensor("out", [G, 128], F32, kind="ExternalOutput")

    with tile.TileContext(nc) as tc:
        with tc.tile_pool(name="sb", bufs=2) as sbuf, \
             tc.tile_pool(name="sb1", bufs=1) as sb1, \
             tc.tile_pool(name="ps", bufs=2, space="PSUM") as psum, \
             tc.tile_pool(name="pp", bufs=1, space="PSUM") as psum1, \
             tc.tile_pool(name="dram", bufs=1, space="DRAM") as dram:
            iota_sb = sb1.tile([128, 32, cwmax], BF16)
            nc.sync.dma_start(iota_sb[:, :, :], iota[:, :, :])
            poolcat = sb1.tile([128, 128], F32)
            for si, s in enumerate("st"):
                pp = pps[s]
                ns = pp['nslot']
                idx_sb = sb1.tile([128, ns // 16], I16, tag=f"idx{s}")
                nc.sync.dma_start(idx_sb[:, :], idx[s][:, :])
                pos_sb = sb1.tile([128, ns // 128], BF16, tag=f"pos{s}")
                nc.sync.dma_start(pos_sb[:, :], pos[s][:, :])
                v_sb = sb1.tile([128, ns // 128], BF16, tag=f"v{s}")
                nc.sync.dma_start(v_sb[:, :], v2[s][:, :])
                wa_sb = sb1.tile([128, 66], BF16, tag=f"wa{s}")
                nc.sync.dma_start(wa_sb[:, :], Wa[s][:, :])
                x2T_sb = sb1.tile([128, NPC], BF16, tag=f"xT{s}")
                nc.sync.dma_start(x2T_sb[:, :], x2T[s][:, :])
                haug = sb1.tile([128, NWIN, 128], BF16, tag=f"ha{s}")
                for w in range(NWIN):
                    m = min(128, NPC - w * 128)
                    ph = psum.tile([128, 66], F32, tag="phd")
                    nc.tensor.matmul(
                        out=ph[0:m, 0:66],
                        lhsT=x2T_sb[:, w * 128:w * 128 + m],
                        rhs=wa_sb[:, 0:66], start=True, stop=True)
                    nc.scalar.activation(haug[0:m, w, 0:66],
                                         ph[0:m, 0:66], AF.Copy)
                nc.vector.memset(haug[:, :, 64:65], 1.0)
                hloc = dram.tile([NPC, 128], BF16, tag=f"hl{s}")
                full = dram.tile([N, 128], BF16, tag=f"hf{s}")
                _store_rows(nc, hloc, haug, 128)
                nc.gpsimd.collective_compute(
                    "AllGather", OP.bypass,
                    replica_groups=[list(range(NC))],
                    ins=[hloc.opt()], outs=[full.opt()])
                x4 = sb1.tile([128, NWIN, 64], BF16, tag=f"x4{s}")
                nc.vector.memset(x4[96:128, NWIN - 1, :], 0.0)
                _edge_phase(nc, sbuf, psum, pp, 128, 65,
                            full[0:SPLIT, :], full[HI_OFF:N, :],
                            idx_sb, pos_sb, v_sb, iota_sb, x4, 64, s)
                ph_sb = sb1.tile([128, NWIN * G], BF16, tag=f"p1h{s}")
                nc.sync.dma_start(ph_sb[:, :], p1hot[s][:, :])
                pl = psum1.tile([128, 64], F32, tag=f"pool{s}")
                for w in range(NWIN):
                    nc.tensor.matmul(
                        out=pl[:, 0:64],
                        lhsT=ph_sb[:, w * G:(w + 1) * G],
                        rhs=x4[:, w, 0:64],
                        start=(w == 0), stop=(w == NWIN - 1))
                nc.vector.tensor_copy(out=poolcat[:, si * 64:si * 64 + 64],
                                      in_=pl[:, 0:64])
            pin = dram.tile([128, 128], F32, tag="pin")
            pout = dram.tile([128, 128], F32, tag="pout")
            nc.sync.dma_start(pin[:, :], poolcat[:, :])
            nc.gpsimd.collective_compute(
                "AllReduce", OP.add, replica_groups=[list(range(NC))],
                ins=[pin.opt()], outs=[pout.opt()])
            pred = sb1.tile([128, 128], F32)
            nc.sync.dma_start(pred[:, :], pout[:, :])
            pg = sb1.tile([128, 64], F32)
            nc.vector.tensor_tensor(out=pg[:, :], in0=pred[:, 0:64],
                                    in1=pred[:, 64:128], op=OP.add)
            id_sb = sb1.tile([128, 128], F32)
            nc.sync.dma_start(id_sb[:, :], ident[:, :])
            pT_ps = psum1.tile([64, 128], F32, tag="pT")
            nc.tensor.transpose(out=pT_ps[:, :], in_=pg[:, :],
                                identity=id_sb[:, :])
            pT = sb1.tile([64, 128], BF16)
            nc.vector.tensor_copy(out=pT[:, :], in_=pT_ps[:, :])
            wl_sb = sb1.tile([64, 128], BF16)
            nc.sync.dma_start(wl_sb[:, :], wlin[:, :])
            oph = psum1.tile([128, 128], F32, tag="pT", name="oph")
            nc.tensor.matmul(out=oph[:, :], lhsT=pT[:, :], rhs=wl_sb[:, :],
                             start=True, stop=True)
            osb = sb1.tile([128, 128], F32)
            nc.scalar.activation(osb[:, :], oph[:, :], AF.Sigmoid)
            nc.sync.dma_start(out[:, :], osb[:, :])
    nc.compile()
    return nc


def kernel(x_s, x_t, edge_index_s, edge_index_t, xs_batch, xt_batch,
           Ws1, as1_src, as1_dst, bs1, Ws2, as2_src, as2_dst, bs2,
           Wt1, at1_src, at1_dst, bt1, Wt2, at2_src, at2_dst, bt2,
           Wlin, blin):
    for b in (bs1, bs2, bt1, bt2, blin):
        assert not np.any(np.asarray(b)), "nonzero bias unsupported"
    x = {"s": np.asarray(x_s, np.float32), "t": np.asarray(x_t, np.float32)}
    W1 = {"s": np.asarray(Ws1, np.float32), "t": np.asarray(Wt1, np.float32)}
    a1s = {"s": np.asarray(as1_src, np.float32),
           "t": np.asarray(at1_src, np.float32)}
    a1d = {"s": np.asarray(as1_dst, np.float32),
           "t": np.asarray(at1_dst, np.float32)}
    W2 = {"s": np.asarray(Ws2, np.float32), "t": np.asarray(Wt2, np.float32)}
    a2s = {"s": np.asarray(as2_src, np.float32),
           "t": np.asarray(at2_src, np.float32)}
    a2d = {"s": np.asarray(as2_dst, np.float32),
           "t": np.asarray(at2_dst, np.float32)}
    batch = {"s": np.asarray(xs_batch), "t": np.asarray(xt_batch)}
    ei = {"s": np.asarray(edge_index_s), "t": np.asarray(edge_index_t)}

    pps = {s: _preprocess(ei[s][0], ei[s][1]) for s in "st"}
    cwmax = max(max(b[1] for b in pps[s]['batches']) for s in "st")
    iota_np = np.ascontiguousarray(np.broadcast_to(
        np.arange(32, dtype=np.float32)[None, :, None],
        (128, 32, cwmax))).astype(BF)

    in_maps1 = []
    for c in range(NC):
        m = {"iota": iota_np}
        for s in "st":
            m[f"xT_{s}"] = np.ascontiguousarray(
                x[s][c * NPC:(c + 1) * NPC, :].T).astype(BF)
            wa = np.zeros((D, 130), np.float32)
            wa[:, 0:128] = W1[s]
            wa[:, 129] = W1[s] @ a1s[s]
            m[f"W1a_{s}"] = wa.astype(BF)
            m[f"idx_{s}"] = _wrap16(pps[s]['idx16'][c])
            m[f"pos_{s}"] = _slot_pc(pps[s]['posrel'][c]).astype(BF)
            sdst = x[s] @ (W1[s] @ a1d[s])
            m[f"v1_{s}"] = _slot_pc(sdst[pps[s]['dstslot'][c]]).astype(BF)
        in_maps1.append(m)

    nc1 = _build_launch1(pps, cwmax)
    res1 = run_bass_kernel_spmd(nc1, in_maps1, core_ids=list(range(NC)),
                                trace=TRACE)
    LAST_EXEC_NS.append(res1.exec_time_ns)
    if TIME_RERUN:
        import time as _t
        t0 = _t.time()
        run_bass_kernel_spmd(nc1, in_maps1, core_ids=list(range(NC)))
        LAST_WALL_S.append(_t.time() - t0)
    x2 = {s: np.concatenate(
        [res1.results[c][f"x2_{s}"].astype(np.float32) for c in range(NC)],
        axis=0) for s in "st"}

    p1hot = {}
    for s in "st":
        cnt = np.maximum(
            np.bincount(batch[s], minlength=G).astype(np.float32), 1.0)
        oh = {}
        for c in range(NC):
            bl = batch[s][c * NPC:(c + 1) * NPC]
            mat = np.zeros((NWIN * 128, G), np.float32)
            mat[np.arange(NPC), bl] = 1.0 / cnt[bl]
            oh[c] = np.ascontiguousarray(
                mat.reshape(NWIN, 128, G).transpose(1, 0, 2)
                .reshape(128, NWIN * G)).astype(BF)
        p1hot[s] = oh

    in_maps2 = []
    for c in range(NC):
        m = {"iota": iota_np,
             "ident": np.eye(128, dtype=np.float32),
             "wlin": np.ascontiguousarray(
                 np.asarray(Wlin, np.float32)[:, c * 128:(c + 1) * 128]
             ).astype(BF)}
        for s in "st":
            m[f"x2T_{s}"] = np.ascontiguousarray(
                x2[s][c * NPC:(c + 1) * NPC, :].T).astype(BF)
            wa = np.zeros((128, 66), np.float32)
            wa[:, 0:64] = W2[s]
            wa[:, 65] = W2[s] @ a2s[s]
            m[f"W2a_{s}"] = wa.astype(BF)
            m[f"idx_{s}"] = _wrap16(pps[s]['idx16'][c])
            m[f"pos_{s}"] = _slot_pc(pps[s]['posrel'][c]).astype(BF)
            sdst2 = x2[s] @ (W2[s] @ a2d[s])
            m[f"v2_{s}"] = _slot_pc(sdst2[pps[s]['dstslot'][c]]).astype(BF)
            m[f"p1hot_{s}"] = p1hot[s][c]
        in_maps2.append(m)

    nc2 = _build_launch2(pps, cwmax)
    res2 = run_bass_kernel_spmd(nc2, in_maps2, core_ids=list(range(NC)),
                                trace=TRACE)
    LAST_EXEC_NS.append(res2.exec_time_ns)
    if TIME_RERUN:
        import time as _t
        t0 = _t.time()
        run_bass_kernel_spmd(nc2, in_maps2, core_ids=list(range(NC)))
        LAST_WALL_S.append(_t.time() - t0)
    out = np.concatenate([res2.results[c]["out"] for c in range(NC)], axis=1)
    return out.astype(np.float32)
